# revision 10
# baseline (speedup 1.0000x reference)
"""Trainium2 Bass kernel for the batched multi-mask de-conv loss problem.

Computes, per (batch, area) over [B=2, A=8192] independent 16x16 areas:
  mc     = differentiable mask-of-interest from mask_combined vs initial_mask_id
  eroded = soft erosion of mc (vertical neighbours only -- or_simple(a,b)=a(2-a)
           makes the horizontal branch algebraically dead)
  me     = eroded * edge_map                      -> output[..., None]
  out2   = var(masked image) * mean(me) * 1000    -> per-area scalar

Sharding: fully data-parallel over B*A = 16384 areas; 2048 areas per core on
8 NeuronCores, SPMD (identical program, different data), no collectives.

Key math identities used (exact up to f32 rounding noise ~1e-6):
  - b = harder_diff_round(mid) == mid exactly for mid in {0,1}
  - eq-select: agree = hdr(a) if b==1 else 1-hdr(a) = |hdr(a) - (1-b)|
  - dr(x - m) = dr(x) - m for integer m; dr(|x|) = |dr(x)| (dr is odd around
    integers) -> the whole per-channel pipeline collapses to 5 chained
    diff_round steps on the raw mask, followed by one flip-subtract.
  - diff_round via a degree-13 odd minimax polynomial of the wrapped residue
    u = x - round(x):  sin(2*pi*x) = u * P(u^2), max err 7.7e-9 -- the
    runtime's ACT table loads hang, so the builtin Sin LUT is unusable and
    sin is evaluated with Square/affine ACT ops + DVE mul/add only.
"""

import numpy as np

import concourse.bass as bass
import concourse.mybir as mybir
from concourse import bacc
from concourse.mybir import AluOpType as Op
from concourse.tile import TileContext

F32 = mybir.dt.float32
MAGIC = float(np.float32(12582912.0))   # 1.5 * 2^23: (x+M)-M == round(x) in f32

# sin(2*pi*u)/(2*pi) = u * P(u^2); P deg-6 minimax on [0, 0.25] (deg-13 in u)
_PC = [1.0, -6.579736232757568, 12.987878799438477, -12.208108901977539,
       6.693719387054443, -2.40122652053833, 0.6022025942802429,
       -0.09926816821098328]

N_CORES = 8
AREAS_TOTAL = 2 * 8192
S = AREAS_TOTAL // N_CORES      # 2048 areas per core
T = S // 128                    # 16 areas per partition
CH = 2                          # areas per partition per chunk
NCHUNK = T // CH                # 8 chunks
PIX = 256                       # 16*16 pixels per area
C = 4                           # mask channels
PAD = 288                       # padded per-area mc stride (16 | 256 | 16)


def _dr_chain(nc, pools, x, width, nsteps):
    """nsteps x  diff_round:  x' = x - sin(2*pi*x)/(2*pi), polynomial form.

    Uses only ops verified to work on this runtime: ACT Square / Copy(scale,
    bias) and DVE tensor_scalar / tensor_tensor.
    """
    xp, ap = pools
    c = _PC
    for _ in range(nsteps):
        # u = x - round(x)  in [-0.5, 0.5]
        r = ap.tile([128, width], F32, tag="sm")
        nc.vector.tensor_scalar(r[:, :], x[:, :], MAGIC, MAGIC, Op.add, Op.subtract)
        u = ap.tile([128, width], F32, tag="sm")
        nc.vector.tensor_tensor(u[:, :], x[:, :], r[:, :], Op.subtract)
        # powers on ACT
        v = ap.tile([128, width], F32, tag="sm")
        nc.scalar.activation(v[:, :], u[:, :], mybir.ActivationFunctionType.Square)
        v2 = ap.tile([128, width], F32, tag="sm")
        nc.scalar.activation(v2[:, :], v[:, :], mybir.ActivationFunctionType.Square)
        v4 = ap.tile([128, width], F32, tag="sm")
        nc.scalar.activation(v4[:, :], v2[:, :], mybir.ActivationFunctionType.Square)
        # affine groups: A=c0+c1 v (ACT), B=c2+c3 v (ACT), D=c4+c5 v (DVE 2x),
        # t2=c6+c7 v (DVE 2x)
        A = ap.tile([128, width], F32, tag="sm")
        nc.scalar.activation(A[:, :], v[:, :], mybir.ActivationFunctionType.Copy,
                             bias=c[0], scale=c[1])
        Bq = ap.tile([128, width], F32, tag="sm")
        nc.scalar.activation(Bq[:, :], v[:, :], mybir.ActivationFunctionType.Copy,
                             bias=c[2], scale=c[3])
        Dq = ap.tile([128, width], F32, tag="sm")
        nc.scalar.activation(Dq[:, :], v[:, :], mybir.ActivationFunctionType.Copy,
                             bias=c[4], scale=c[5])
        t2 = ap.tile([128, width], F32, tag="sm")
        nc.scalar.activation(t2[:, :], v[:, :], mybir.ActivationFunctionType.Copy,
                             bias=c[6], scale=c[7])
        # P = (A + B*v2) + v4*(D + t2*v2)
        # (GpSimd offload of the D-branch hangs this runtime -- POOL
        # tensor ops appear to need ucode the fake_nrt shim doesn't load --
        # so everything tensor-tensor stays on the vector engine.)
        t1 = ap.tile([128, width], F32, tag="sm")
        nc.vector.tensor_tensor(t1[:, :], Bq[:, :], v2[:, :], Op.mult)
        E = ap.tile([128, width], F32, tag="sm")
        nc.vector.tensor_tensor(E[:, :], A[:, :], t1[:, :], Op.add)
        t3 = ap.tile([128, width], F32, tag="sm")
        nc.vector.tensor_tensor(t3[:, :], t2[:, :], v2[:, :], Op.mult)
        Fq = ap.tile([128, width], F32, tag="sm")
        nc.vector.tensor_tensor(Fq[:, :], Dq[:, :], t3[:, :], Op.add)
        G = ap.tile([128, width], F32, tag="sm")
        nc.vector.tensor_tensor(G[:, :], Fq[:, :], v4[:, :], Op.mult)
        P = ap.tile([128, width], F32, tag="sm")
        nc.vector.tensor_tensor(P[:, :], E[:, :], G[:, :], Op.add)
        sP = ap.tile([128, width], F32, tag="sm")
        nc.vector.tensor_tensor(sP[:, :], u[:, :], P[:, :], Op.mult)
        xn = xp.tile([128, width], F32, tag=f"x{width}")
        nc.vector.tensor_tensor(xn[:, :], x[:, :], sP[:, :], Op.subtract)
        x = xn
    return x


def build_nc():
    nc = bacc.Bacc("TRN2", target_bir_lowering=False, debug=False)

    mask_d = nc.dram_tensor("mask", [S, PIX * C], F32, kind="ExternalInput")
    img_d = nc.dram_tensor("img", [S, PIX], F32, kind="ExternalInput")
    edge_d = nc.dram_tensor("edge", [S, PIX], F32, kind="ExternalInput")
    mid_d = nc.dram_tensor("mid", [S, C], F32, kind="ExternalInput")
    me_d = nc.dram_tensor("me", [S, PIX], F32, kind="ExternalOutput")
    out2_d = nc.dram_tensor("out2", [S], F32, kind="ExternalOutput")

    # DRAM views with partition p <-> area p*T + t
    mask_v = mask_d.ap().rearrange("(p t) f -> p t f", p=128)     # [128, 16, 1024]
    img_v = img_d.ap().rearrange("(p t) f -> p t f", p=128)
    edge_v = edge_d.ap().rearrange("(p t) f -> p t f", p=128)
    mid_v = mid_d.ap().rearrange("(p t) c -> p (t c)", p=128)     # [128, 64]
    me_v = me_d.ap().rearrange("(p t) f -> p t f", p=128)
    out2_v = out2_d.ap().rearrange("(p t) -> p t", p=128)         # [128, 16]

    with TileContext(nc) as tc:
        with (
            tc.tile_pool(name="xp", bufs=3) as xp,          # dr-chain ping-pong
            tc.tile_pool(name="ap", bufs=12) as ap,         # poly scratch
            tc.tile_pool(name="zp", bufs=1) as zp,
            tc.tile_pool(name="iop", bufs=2) as iop,        # img/edge/me
            tc.tile_pool(name="smp", bufs=8) as smp,        # small work tiles
            tc.tile_pool(name="mcp", bufs=2) as mcpool,     # padded mc
            tc.tile_pool(name="stp", bufs=1) as stp,        # persistent stats
        ):
            BIG = CH * PIX * C          # 2048
            SMALL = CH * PIX            # 512

            # persistent stats tiles [128, T]
            s_mc = stp.tile([128, T], F32, tag="s_mc")
            s_mi = stp.tile([128, T], F32, tag="s_mi")
            s_me = stp.tile([128, T], F32, tag="s_me")
            s_e2 = stp.tile([128, T], F32, tag="s_e2")
            inv_t = stp.tile([128, T], F32, tag="inv")
            meann_t = stp.tile([128, T], F32, tag="meann")
            denom_t = stp.tile([128, T], F32, tag="denom")

            # mid -> m = 1 - mid, once for the whole core
            mid_t = stp.tile([128, T * C], F32, tag="mid")
            nc.sync.dma_start(mid_t[:, :], mid_v)
            m_t = stp.tile([128, T * C], F32, tag="m")
            nc.vector.tensor_scalar(m_t[:, :], mid_t[:, :], -1.0, 1.0, Op.mult, Op.add)

            for c in range(NCHUNK):
                tsl = slice(c * CH, (c + 1) * CH)

                # ---- loads ----
                x = xp.tile([128, BIG], F32, tag=f"x{BIG}")
                nc.sync.dma_start(
                    x[:, :].rearrange("p (t f) -> p t f", t=CH), mask_v[:, tsl, :]
                )
                img_t = iop.tile([128, SMALL], F32, tag="img")
                nc.sync.dma_start(
                    img_t[:, :].rearrange("p (t f) -> p t f", t=CH), img_v[:, tsl, :]
                )
                edge_t = iop.tile([128, SMALL], F32, tag="edge")
                nc.sync.dma_start(
                    edge_t[:, :].rearrange("p (t f) -> p t f", t=CH), edge_v[:, tsl, :]
                )

                # ---- 5 chained diff_round steps on [128, 2048] ----
                x = _dr_chain(nc, (xp, ap), x, BIG, 5)

                # ---- flip-subtract: z = x5 - m  (m broadcast over pixels) ----
                m_b = (
                    m_t[:, c * CH * C:(c + 1) * CH * C]
                    .rearrange("p (a c) -> p a c", c=C)
                    .unsqueeze(2)
                    .to_broadcast([128, CH, PIX, C])
                )
                z = zp.tile([128, BIG], F32, tag="z")
                z4 = z[:, :].rearrange("p (a x c) -> p a x c", a=CH, c=C)
                x4 = x[:, :].rearrange("p (a x c) -> p a x c", a=CH, c=C)
                nc.vector.tensor_tensor(z4, x4, m_b, Op.subtract)

                # ---- channel-pair products, abs via sign-bit clear ----
                pa = smp.tile([128, SMALL], F32, tag="tmp")
                pb = smp.tile([128, SMALL], F32, tag="tmp")
                pa3 = pa[:, :].rearrange("p (a x) -> p a x", a=CH).unsqueeze(3)
                pb3 = pb[:, :].rearrange("p (a x) -> p a x", a=CH).unsqueeze(3)
                nc.vector.tensor_tensor(pa3, z4[:, :, :, 0:1], z4[:, :, :, 1:2], Op.mult)
                nc.vector.tensor_tensor(pb3, z4[:, :, :, 2:3], z4[:, :, :, 3:4], Op.mult)
                pa_u = pa[:, :].bitcast(mybir.dt.uint32)
                pb_u = pb[:, :].bitcast(mybir.dt.uint32)
                nc.vector.tensor_scalar(pa_u, pa_u, 0x7FFFFFFF, None, Op.bitwise_and)
                nc.vector.tensor_scalar(pb_u, pb_u, 0x7FFFFFFF, None, Op.bitwise_and)

                # ---- one more diff_round on each product ----
                wa = _dr_chain(nc, (smp, ap), pa, SMALL, 1)
                wb = _dr_chain(nc, (smp, ap), pb, SMALL, 1)

                # ---- mc = wa*wb into padded tile ----
                mcp = mcpool.tile([128, CH * PAD], F32, tag="mcp")
                mcp3 = mcp[:, :].rearrange("p (a k) -> p a k", a=CH)
                nc.vector.memset(mcp3[:, :, 0:16], 0.0)
                nc.vector.memset(mcp3[:, :, 272:288], 0.0)
                nc.vector.tensor_tensor(
                    mcp3[:, :, 16:272],
                    wa[:, :].rearrange("p (a x) -> p a x", a=CH),
                    wb[:, :].rearrange("p (a x) -> p a x", a=CH),
                    Op.mult,
                )
                nc.vector.tensor_reduce(
                    s_mc[:, tsl], mcp3[:, :, 16:272], axis=mybir.AxisListType.X,
                    op=Op.add,
                )

                mc_v = mcp3[:, :, 16:272]     # [128, CH, 256]
                up_v = mcp3[:, :, 32:288]     # mc[i+16], zero past bottom row
                dn_v = mcp3[:, :, 0:256]      # mc[i-16], zero before top row

                # ---- erosion: ev = up + dn - 2*up*dn ; r = 1 - ev*mc ----
                t1 = smp.tile([128, SMALL], F32, tag="tmp")
                p1 = smp.tile([128, SMALL], F32, tag="tmp")
                t13 = t1[:, :].rearrange("p (a x) -> p a x", a=CH)
                p13 = p1[:, :].rearrange("p (a x) -> p a x", a=CH)
                nc.vector.tensor_tensor(t13, up_v, dn_v, Op.add)
                nc.vector.tensor_tensor(p13, up_v, dn_v, Op.mult)
                p2 = smp.tile([128, SMALL], F32, tag="tmp")
                nc.vector.tensor_scalar(p2[:, :], p1[:, :], 2.0, None, Op.mult)
                ev = smp.tile([128, SMALL], F32, tag="tmp")
                nc.vector.tensor_tensor(ev[:, :], t1[:, :], p2[:, :], Op.subtract)
                q = smp.tile([128, SMALL], F32, tag="tmp")
                q3 = q[:, :].rearrange("p (a x) -> p a x", a=CH)
                nc.vector.tensor_tensor(
                    q3, ev[:, :].rearrange("p (a x) -> p a x", a=CH), mc_v, Op.mult)
                r = smp.tile([128, SMALL], F32, tag="tmp")
                nc.vector.tensor_scalar(r[:, :], q[:, :], -1.0, 1.0, Op.mult, Op.add)
                r2 = smp.tile([128, SMALL], F32, tag="tmp")
                nc.scalar.activation(r2[:, :], r[:, :],
                                     mybir.ActivationFunctionType.Square)

                # ---- masked edges me = r2 * (mc*edge) ----
                metmp = smp.tile([128, SMALL], F32, tag="tmp")
                me3 = metmp[:, :].rearrange("p (a x) -> p a x", a=CH)
                nc.vector.tensor_tensor(
                    me3, mc_v, edge_t[:, :].rearrange("p (a x) -> p a x", a=CH),
                    Op.mult)
                me_t = iop.tile([128, SMALL], F32, tag="me")
                nc.vector.tensor_tensor(me_t[:, :], r2[:, :], metmp[:, :], Op.mult)
                nc.vector.tensor_reduce(
                    s_me[:, tsl],
                    me_t[:, :].rearrange("p (a x) -> p a x", a=CH),
                    axis=mybir.AxisListType.X, op=Op.add,
                )

                # ---- masked image mi = mc * img ----
                mi = smp.tile([128, SMALL], F32, tag="tmp")
                mi3 = mi[:, :].rearrange("p (a x) -> p a x", a=CH)
                nc.vector.tensor_tensor(
                    mi3, mc_v, img_t[:, :].rearrange("p (a x) -> p a x", a=CH),
                    Op.mult)
                nc.vector.tensor_reduce(
                    s_mi[:, tsl], mi3, axis=mybir.AxisListType.X, op=Op.add,
                )

                # ---- per-chunk stats: denom, 1/denom, mean ----
                csl = tsl
                nc.vector.tensor_scalar(denom_t[:, csl], s_mc[:, csl], 1e-8, None,
                                        Op.add)
                nc.vector.reciprocal(inv_t[:, csl], denom_t[:, csl])
                nc.vector.tensor_tensor(meann_t[:, csl], s_mi[:, csl], inv_t[:, csl],
                                        Op.mult)

                # ---- variance accumulation ----
                d_t = smp.tile([128, SMALL], F32, tag="tmp")
                for j in range(CH):
                    col = c * CH + j
                    jsl = slice(j * PIX, (j + 1) * PIX)
                    nc.vector.tensor_scalar(
                        d_t[:, jsl], mi[:, jsl], meann_t[:, col:col + 1], None,
                        Op.subtract)
                e_t = smp.tile([128, SMALL], F32, tag="tmp")
                e3 = e_t[:, :].rearrange("p (a x) -> p a x", a=CH)
                nc.vector.tensor_tensor(
                    e3, d_t[:, :].rearrange("p (a x) -> p a x", a=CH), mc_v, Op.mult)
                e2_t = smp.tile([128, SMALL], F32, tag="tmp")
                nc.scalar.activation(e2_t[:, :], e_t[:, :],
                                     mybir.ActivationFunctionType.Square)
                nc.vector.tensor_reduce(
                    s_e2[:, tsl],
                    e2_t[:, :].rearrange("p (a x) -> p a x", a=CH),
                    axis=mybir.AxisListType.X, op=Op.add,
                )

                # ---- store masked edges ----
                nc.sync.dma_start(
                    me_v[:, tsl, :], me_t[:, :].rearrange("p (t f) -> p t f", t=CH)
                )

            # ---- final per-area scalar: out2 = (se2/denom) * (sme/256) * 1000
            varr = stp.tile([128, T], F32, tag="varr")
            nc.vector.tensor_tensor(varr[:, :], s_e2[:, :], inv_t[:, :], Op.mult)
            lss = stp.tile([128, T], F32, tag="lss")
            nc.vector.tensor_scalar(lss[:, :], s_me[:, :], 1000.0 / 256.0, None,
                                    Op.mult)
            o2 = stp.tile([128, T], F32, tag="o2")
            nc.vector.tensor_tensor(o2[:, :], varr[:, :], lss[:, :], Op.mult)
            nc.sync.dma_start(out2_v, o2[:, :])

    nc.compile()
    return nc


_NC_CACHE = None


def _get_nc():
    global _NC_CACHE
    if _NC_CACHE is None:
        _NC_CACHE = build_nc()
    return _NC_CACHE


def kernel(resized_image, mask_combined, initial_mask_id, edge_map,
           _results_hook=None):
    from concourse.bass_utils import run_bass_kernel_spmd

    B, A = 2, 8192
    mask_f = np.ascontiguousarray(
        np.asarray(mask_combined, dtype=np.float32).reshape(B * A, PIX * C))
    img_f = np.ascontiguousarray(
        np.asarray(resized_image, dtype=np.float32).reshape(B * A, PIX))
    edge_f = np.ascontiguousarray(
        np.asarray(edge_map, dtype=np.float32).reshape(B * A, PIX))
    mid_f = np.ascontiguousarray(
        np.asarray(initial_mask_id, dtype=np.float32).reshape(B * A, C))

    nc = _get_nc()
    in_maps = [
        {
            "mask": mask_f[i * S:(i + 1) * S],
            "img": img_f[i * S:(i + 1) * S],
            "edge": edge_f[i * S:(i + 1) * S],
            "mid": mid_f[i * S:(i + 1) * S],
        }
        for i in range(N_CORES)
    ]
    res = run_bass_kernel_spmd(nc, in_maps, core_ids=list(range(N_CORES)))
    if _results_hook is not None:
        _results_hook(res)

    me = np.concatenate([r["me"] for r in res.results], axis=0)
    out2 = np.concatenate([r["out2"] for r in res.results], axis=0)
    return (
        me.reshape(B, A, 16, 16, 1).astype(np.float32),
        out2.reshape(B, A).astype(np.float32),
    )


# revision 17
# speedup vs baseline: 1.1684x; 1.1684x over previous
"""Trainium2 Bass kernel for the batched multi-mask de-conv loss problem.

Computes, per (batch, area) over [B=2, A=8192] independent 16x16 areas:
  mc     = differentiable mask-of-interest from mask_combined vs initial_mask_id
  eroded = soft erosion of mc (vertical neighbours only -- or_simple(a,b)=a(2-a)
           makes the horizontal branch algebraically dead)
  me     = eroded * edge_map                      -> output[..., None]
  out2   = var(masked image) * mean(me) * 1000    -> per-area scalar

Sharding: fully data-parallel over B*A = 16384 areas; 2048 areas per core on
8 NeuronCores, SPMD (identical program, different data), no collectives.

Key math identities used (exact up to f32 rounding noise ~1e-6):
  - b = harder_diff_round(mid) == mid exactly for mid in {0,1}
  - eq-select: agree = hdr(a) if b==1 else 1-hdr(a) = |hdr(a) - (1-b)|
  - dr(x - m) = dr(x) - m for integer m; dr(|x|) = |dr(x)| (dr is odd around
    integers) -> the whole per-channel pipeline collapses to 5 chained
    diff_round steps on the raw mask, followed by one flip-subtract.
  - diff_round via a degree-11 odd minimax polynomial of the wrapped residue
    u = x - round(x):  sin(2*pi*x) = u * P(u^2), max err 5.9e-7 (at the f32
    noise floor of the chain) -- the runtime's ACT table loads hang, so the
    builtin Sin LUT is unusable and sin is evaluated with Square/affine ACT
    ops + DVE mul/add only.  dr chains of chunk pairs are emitted step-major
    to interleave one chunk's ACT phase with the other's DVE phase.
"""

import numpy as np

import concourse.bass as bass
import concourse.mybir as mybir
from concourse import bacc
from concourse.mybir import AluOpType as Op
from concourse.tile import TileContext

F32 = mybir.dt.float32
MAGIC = float(np.float32(12582912.0))   # 1.5 * 2^23: (x+M)-M == round(x) in f32

# sin(2*pi*u)/(2*pi) = u * P(u^2); P deg-5 minimax on [0, 0.25] (deg-11 in u,
# max err 5.9e-7 -- at the f32 noise floor of the 5-step chain)
_PC = [0.9999999403953552, -6.5797224044799805, 12.987188339233398,
       -12.195494651794434, 6.589564323425293, -2.001596689224243]

N_CORES = 8
AREAS_TOTAL = 2 * 8192
S = AREAS_TOTAL // N_CORES      # 2048 areas per core
T = S // 128                    # 16 areas per partition
CH = 2                          # areas per partition per chunk
NCHUNK = T // CH                # 8 chunks
PIX = 256                       # 16*16 pixels per area
C = 4                           # mask channels
PAD = 288                       # padded per-area mc stride (16 | 256 | 16)


def _dr_chain_multi(nc, pools, xs, width, nsteps):
    """nsteps x diff_round on a LIST of tiles, emitted step-major so the
    Tile scheduler overlaps one tile's ACT phase with another's DVE phase."""
    for _ in range(nsteps):
        xs = [_dr_step(nc, pools, x, width) for x in xs]
    return xs


def _dr_chain(nc, pools, x, width, nsteps):
    for _ in range(nsteps):
        x = _dr_step(nc, pools, x, width)
    return x


def _dr_step(nc, pools, x, width):
    """One diff_round:  x' = x - sin(2*pi*x)/(2*pi), polynomial form.

    Uses only ops verified to work on this runtime: ACT Square / Copy(scale,
    bias) and DVE tensor_scalar / tensor_tensor.
    """
    xp, ap = pools
    c = _PC
    if True:
        # u = x - round(x)  in [-0.5, 0.5]
        r = ap.tile([128, width], F32, tag="sm")
        nc.vector.tensor_scalar(r[:, :], x[:, :], MAGIC, MAGIC, Op.add, Op.subtract)
        u = ap.tile([128, width], F32, tag="sm")
        nc.vector.tensor_tensor(u[:, :], x[:, :], r[:, :], Op.subtract)
        # powers on ACT
        v = ap.tile([128, width], F32, tag="sm")
        nc.scalar.activation(v[:, :], u[:, :], mybir.ActivationFunctionType.Square)
        v2 = ap.tile([128, width], F32, tag="sm")
        nc.scalar.activation(v2[:, :], v[:, :], mybir.ActivationFunctionType.Square)
        v4 = ap.tile([128, width], F32, tag="sm")
        nc.scalar.activation(v4[:, :], v2[:, :], mybir.ActivationFunctionType.Square)
        # affine groups on ACT: A=c0+c1 v, B=c2+c3 v, D=c4+c5 v
        # (GpSimd offload hangs this runtime -- POOL tensor ops appear to
        # need ucode the fake_nrt shim doesn't load -- so tensor-tensor work
        # stays on the vector engine and single-input affines go to ACT.)
        A = ap.tile([128, width], F32, tag="sm")
        nc.scalar.activation(A[:, :], v[:, :], mybir.ActivationFunctionType.Copy,
                             bias=c[0], scale=c[1])
        Bq = ap.tile([128, width], F32, tag="sm")
        nc.scalar.activation(Bq[:, :], v[:, :], mybir.ActivationFunctionType.Copy,
                             bias=c[2], scale=c[3])
        Dq = ap.tile([128, width], F32, tag="sm")
        nc.scalar.activation(Dq[:, :], v[:, :], mybir.ActivationFunctionType.Copy,
                             bias=c[4], scale=c[5])
        # P = (A + B*v2) + D*v4
        t1 = ap.tile([128, width], F32, tag="sm")
        nc.vector.tensor_tensor(t1[:, :], Bq[:, :], v2[:, :], Op.mult)
        t4 = ap.tile([128, width], F32, tag="sm")
        nc.vector.tensor_tensor(t4[:, :], Dq[:, :], v4[:, :], Op.mult)
        E = ap.tile([128, width], F32, tag="sm")
        nc.vector.tensor_tensor(E[:, :], A[:, :], t1[:, :], Op.add)
        P = ap.tile([128, width], F32, tag="sm")
        nc.vector.tensor_tensor(P[:, :], E[:, :], t4[:, :], Op.add)
        sP = ap.tile([128, width], F32, tag="sm")
        nc.vector.tensor_tensor(sP[:, :], u[:, :], P[:, :], Op.mult)
        xn = xp.tile([128, width], F32, tag=f"x{width}")
        nc.vector.tensor_tensor(xn[:, :], x[:, :], sP[:, :], Op.subtract)
    return xn


def build_nc():
    nc = bacc.Bacc("TRN2", target_bir_lowering=False, debug=False)

    mask_d = nc.dram_tensor("mask", [S, PIX * C], F32, kind="ExternalInput")
    img_d = nc.dram_tensor("img", [S, PIX], F32, kind="ExternalInput")
    edge_d = nc.dram_tensor("edge", [S, PIX], F32, kind="ExternalInput")
    mid_d = nc.dram_tensor("mid", [S, C], F32, kind="ExternalInput")
    me_d = nc.dram_tensor("me", [S, PIX], F32, kind="ExternalOutput")
    out2_d = nc.dram_tensor("out2", [S], F32, kind="ExternalOutput")

    # DRAM views with partition p <-> area p*T + t
    mask_v = mask_d.ap().rearrange("(p t) f -> p t f", p=128)     # [128, 16, 1024]
    img_v = img_d.ap().rearrange("(p t) f -> p t f", p=128)
    edge_v = edge_d.ap().rearrange("(p t) f -> p t f", p=128)
    mid_v = mid_d.ap().rearrange("(p t) c -> p (t c)", p=128)     # [128, 64]
    me_v = me_d.ap().rearrange("(p t) f -> p t f", p=128)
    out2_v = out2_d.ap().rearrange("(p t) -> p t", p=128)         # [128, 16]

    with TileContext(nc) as tc:
        with (
            tc.tile_pool(name="xp", bufs=5) as xp,          # dr-chain ping-pong
            tc.tile_pool(name="ap", bufs=12) as ap,         # poly scratch
            tc.tile_pool(name="zp", bufs=1) as zp,
            tc.tile_pool(name="iop", bufs=2) as iop,        # img/edge/me
            tc.tile_pool(name="smp", bufs=8) as smp,        # small work tiles
            tc.tile_pool(name="mcp", bufs=2) as mcpool,     # padded mc
            tc.tile_pool(name="stp", bufs=1) as stp,        # persistent stats
        ):
            BIG = CH * PIX * C          # 2048
            SMALL = CH * PIX            # 512

            # persistent stats tiles [128, T]
            s_mc = stp.tile([128, T], F32, tag="s_mc")
            s_mi = stp.tile([128, T], F32, tag="s_mi")
            s_me = stp.tile([128, T], F32, tag="s_me")
            s_e2 = stp.tile([128, T], F32, tag="s_e2")
            inv_t = stp.tile([128, T], F32, tag="inv")
            meann_t = stp.tile([128, T], F32, tag="meann")
            denom_t = stp.tile([128, T], F32, tag="denom")

            # mid -> m = 1 - mid, once for the whole core
            mid_t = stp.tile([128, T * C], F32, tag="mid")
            nc.sync.dma_start(mid_t[:, :], mid_v)
            m_t = stp.tile([128, T * C], F32, tag="m")
            nc.vector.tensor_scalar(m_t[:, :], mid_t[:, :], -1.0, 1.0, Op.mult, Op.add)

            for cpair in range(NCHUNK // 2):
              pair = (2 * cpair, 2 * cpair + 1)
              xs = []
              for c in pair:
                tsl = slice(c * CH, (c + 1) * CH)
                x = xp.tile([128, BIG], F32, tag=f"x{BIG}")
                nc.sync.dma_start(
                    x[:, :].rearrange("p (t f) -> p t f", t=CH), mask_v[:, tsl, :]
                )
                xs.append(x)
              # ---- 5 chained diff_round steps, pair-interleaved ----
              xs = _dr_chain_multi(nc, (xp, ap), xs, BIG, 5)
              for c, x in zip(pair, xs):
                tsl = slice(c * CH, (c + 1) * CH)
                img_t = iop.tile([128, SMALL], F32, tag="img")
                nc.sync.dma_start(
                    img_t[:, :].rearrange("p (t f) -> p t f", t=CH), img_v[:, tsl, :]
                )
                edge_t = iop.tile([128, SMALL], F32, tag="edge")
                nc.sync.dma_start(
                    edge_t[:, :].rearrange("p (t f) -> p t f", t=CH), edge_v[:, tsl, :]
                )

                # ---- flip-subtract: z = x5 - m  (m broadcast over pixels) ----
                m_b = (
                    m_t[:, c * CH * C:(c + 1) * CH * C]
                    .rearrange("p (a c) -> p a c", c=C)
                    .unsqueeze(2)
                    .to_broadcast([128, CH, PIX, C])
                )
                z = zp.tile([128, BIG], F32, tag="z")
                z4 = z[:, :].rearrange("p (a x c) -> p a x c", a=CH, c=C)
                x4 = x[:, :].rearrange("p (a x c) -> p a x c", a=CH, c=C)
                nc.vector.tensor_tensor(z4, x4, m_b, Op.subtract)

                # ---- channel-pair products, abs via sign-bit clear ----
                pa = smp.tile([128, SMALL], F32, tag="tmp")
                pb = smp.tile([128, SMALL], F32, tag="tmp")
                pa3 = pa[:, :].rearrange("p (a x) -> p a x", a=CH).unsqueeze(3)
                pb3 = pb[:, :].rearrange("p (a x) -> p a x", a=CH).unsqueeze(3)
                nc.vector.tensor_tensor(pa3, z4[:, :, :, 0:1], z4[:, :, :, 1:2], Op.mult)
                nc.vector.tensor_tensor(pb3, z4[:, :, :, 2:3], z4[:, :, :, 3:4], Op.mult)
                pa_u = pa[:, :].bitcast(mybir.dt.uint32)
                pb_u = pb[:, :].bitcast(mybir.dt.uint32)
                nc.vector.tensor_scalar(pa_u, pa_u, 0x7FFFFFFF, None, Op.bitwise_and)
                nc.vector.tensor_scalar(pb_u, pb_u, 0x7FFFFFFF, None, Op.bitwise_and)

                # ---- one more diff_round on each product ----
                wa = _dr_chain(nc, (smp, ap), pa, SMALL, 1)
                wb = _dr_chain(nc, (smp, ap), pb, SMALL, 1)

                # ---- mc = wa*wb into padded tile ----
                mcp = mcpool.tile([128, CH * PAD], F32, tag="mcp")
                mcp3 = mcp[:, :].rearrange("p (a k) -> p a k", a=CH)
                nc.vector.memset(mcp3[:, :, 0:16], 0.0)
                nc.vector.memset(mcp3[:, :, 272:288], 0.0)
                nc.vector.tensor_tensor(
                    mcp3[:, :, 16:272],
                    wa[:, :].rearrange("p (a x) -> p a x", a=CH),
                    wb[:, :].rearrange("p (a x) -> p a x", a=CH),
                    Op.mult,
                )
                nc.vector.tensor_reduce(
                    s_mc[:, tsl], mcp3[:, :, 16:272], axis=mybir.AxisListType.X,
                    op=Op.add,
                )

                mc_v = mcp3[:, :, 16:272]     # [128, CH, 256]
                up_v = mcp3[:, :, 32:288]     # mc[i+16], zero past bottom row
                dn_v = mcp3[:, :, 0:256]      # mc[i-16], zero before top row

                # ---- erosion: ev = up + dn - 2*up*dn ; r = 1 - ev*mc ----
                t1 = smp.tile([128, SMALL], F32, tag="tmp")
                p1 = smp.tile([128, SMALL], F32, tag="tmp")
                t13 = t1[:, :].rearrange("p (a x) -> p a x", a=CH)
                p13 = p1[:, :].rearrange("p (a x) -> p a x", a=CH)
                nc.vector.tensor_tensor(t13, up_v, dn_v, Op.add)
                nc.vector.tensor_tensor(p13, up_v, dn_v, Op.mult)
                p2 = smp.tile([128, SMALL], F32, tag="tmp")
                nc.vector.tensor_scalar(p2[:, :], p1[:, :], 2.0, None, Op.mult)
                ev = smp.tile([128, SMALL], F32, tag="tmp")
                nc.vector.tensor_tensor(ev[:, :], t1[:, :], p2[:, :], Op.subtract)
                q = smp.tile([128, SMALL], F32, tag="tmp")
                q3 = q[:, :].rearrange("p (a x) -> p a x", a=CH)
                nc.vector.tensor_tensor(
                    q3, ev[:, :].rearrange("p (a x) -> p a x", a=CH), mc_v, Op.mult)
                r = smp.tile([128, SMALL], F32, tag="tmp")
                nc.vector.tensor_scalar(r[:, :], q[:, :], -1.0, 1.0, Op.mult, Op.add)
                r2 = smp.tile([128, SMALL], F32, tag="tmp")
                nc.scalar.activation(r2[:, :], r[:, :],
                                     mybir.ActivationFunctionType.Square)

                # ---- masked edges me = r2 * (mc*edge) ----
                metmp = smp.tile([128, SMALL], F32, tag="tmp")
                me3 = metmp[:, :].rearrange("p (a x) -> p a x", a=CH)
                nc.vector.tensor_tensor(
                    me3, mc_v, edge_t[:, :].rearrange("p (a x) -> p a x", a=CH),
                    Op.mult)
                me_t = iop.tile([128, SMALL], F32, tag="me")
                nc.vector.tensor_tensor(me_t[:, :], r2[:, :], metmp[:, :], Op.mult)
                nc.vector.tensor_reduce(
                    s_me[:, tsl],
                    me_t[:, :].rearrange("p (a x) -> p a x", a=CH),
                    axis=mybir.AxisListType.X, op=Op.add,
                )

                # ---- masked image mi = mc * img ----
                mi = smp.tile([128, SMALL], F32, tag="tmp")
                mi3 = mi[:, :].rearrange("p (a x) -> p a x", a=CH)
                nc.vector.tensor_tensor(
                    mi3, mc_v, img_t[:, :].rearrange("p (a x) -> p a x", a=CH),
                    Op.mult)
                nc.vector.tensor_reduce(
                    s_mi[:, tsl], mi3, axis=mybir.AxisListType.X, op=Op.add,
                )

                # ---- per-chunk stats: denom, 1/denom, mean ----
                csl = tsl
                nc.vector.tensor_scalar(denom_t[:, csl], s_mc[:, csl], 1e-8, None,
                                        Op.add)
                nc.vector.reciprocal(inv_t[:, csl], denom_t[:, csl])
                nc.vector.tensor_tensor(meann_t[:, csl], s_mi[:, csl], inv_t[:, csl],
                                        Op.mult)

                # ---- variance accumulation ----
                d_t = smp.tile([128, SMALL], F32, tag="tmp")
                for j in range(CH):
                    col = c * CH + j
                    jsl = slice(j * PIX, (j + 1) * PIX)
                    nc.vector.tensor_scalar(
                        d_t[:, jsl], mi[:, jsl], meann_t[:, col:col + 1], None,
                        Op.subtract)
                e_t = smp.tile([128, SMALL], F32, tag="tmp")
                e3 = e_t[:, :].rearrange("p (a x) -> p a x", a=CH)
                nc.vector.tensor_tensor(
                    e3, d_t[:, :].rearrange("p (a x) -> p a x", a=CH), mc_v, Op.mult)
                e2_t = smp.tile([128, SMALL], F32, tag="tmp")
                nc.scalar.activation(e2_t[:, :], e_t[:, :],
                                     mybir.ActivationFunctionType.Square)
                nc.vector.tensor_reduce(
                    s_e2[:, tsl],
                    e2_t[:, :].rearrange("p (a x) -> p a x", a=CH),
                    axis=mybir.AxisListType.X, op=Op.add,
                )

                # ---- store masked edges ----
                nc.sync.dma_start(
                    me_v[:, tsl, :], me_t[:, :].rearrange("p (t f) -> p t f", t=CH)
                )

            # ---- final per-area scalar: out2 = (se2/denom) * (sme/256) * 1000
            varr = stp.tile([128, T], F32, tag="varr")
            nc.vector.tensor_tensor(varr[:, :], s_e2[:, :], inv_t[:, :], Op.mult)
            lss = stp.tile([128, T], F32, tag="lss")
            nc.vector.tensor_scalar(lss[:, :], s_me[:, :], 1000.0 / 256.0, None,
                                    Op.mult)
            o2 = stp.tile([128, T], F32, tag="o2")
            nc.vector.tensor_tensor(o2[:, :], varr[:, :], lss[:, :], Op.mult)
            nc.sync.dma_start(out2_v, o2[:, :])

    nc.compile()
    return nc


_NC_CACHE = None


def _get_nc():
    global _NC_CACHE
    if _NC_CACHE is None:
        _NC_CACHE = build_nc()
    return _NC_CACHE


def kernel(resized_image, mask_combined, initial_mask_id, edge_map,
           _results_hook=None):
    from concourse.bass_utils import run_bass_kernel_spmd

    B, A = 2, 8192
    mask_f = np.ascontiguousarray(
        np.asarray(mask_combined, dtype=np.float32).reshape(B * A, PIX * C))
    img_f = np.ascontiguousarray(
        np.asarray(resized_image, dtype=np.float32).reshape(B * A, PIX))
    edge_f = np.ascontiguousarray(
        np.asarray(edge_map, dtype=np.float32).reshape(B * A, PIX))
    mid_f = np.ascontiguousarray(
        np.asarray(initial_mask_id, dtype=np.float32).reshape(B * A, C))

    nc = _get_nc()
    in_maps = [
        {
            "mask": mask_f[i * S:(i + 1) * S],
            "img": img_f[i * S:(i + 1) * S],
            "edge": edge_f[i * S:(i + 1) * S],
            "mid": mid_f[i * S:(i + 1) * S],
        }
        for i in range(N_CORES)
    ]
    res = run_bass_kernel_spmd(nc, in_maps, core_ids=list(range(N_CORES)))
    if _results_hook is not None:
        _results_hook(res)

    me = np.concatenate([r["me"] for r in res.results], axis=0)
    out2 = np.concatenate([r["out2"] for r in res.results], axis=0)
    return (
        me.reshape(B, A, 16, 16, 1).astype(np.float32),
        out2.reshape(B, A).astype(np.float32),
    )


# revision 23
# speedup vs baseline: 1.3704x; 1.1729x over previous
"""Trainium2 Bass kernel for the batched multi-mask de-conv loss problem.

Computes, per (batch, area) over [B=2, A=8192] independent 16x16 areas:
  mc     = differentiable mask-of-interest from mask_combined vs initial_mask_id
  eroded = soft erosion of mc (vertical neighbours only -- or_simple(a,b)=a(2-a)
           makes the horizontal branch algebraically dead)
  me     = eroded * edge_map                      -> output[..., None]
  out2   = var(masked image) * mean(me) * 1000    -> per-area scalar

Sharding: fully data-parallel over B*A = 16384 areas; 2048 areas per core on
8 NeuronCores, SPMD (identical program, different data), no collectives.

Key math identities used (exact up to f32 rounding noise ~1e-6):
  - b = harder_diff_round(mid) == mid exactly for mid in {0,1}
  - eq-select: agree = hdr(a) if b==1 else 1-hdr(a) = |hdr(a) - (1-b)|
  - dr(x - m) = dr(x) - m for integer m; dr(|x|) = |dr(x)| (dr is odd around
    integers) -> the whole per-channel pipeline collapses to 5 chained
    diff_round steps on the raw mask, followed by one flip-subtract.
  - diff_round via a degree-11 odd minimax polynomial of the wrapped residue
    u = x - round(x):  sin(2*pi*x) = u * P(u^2), max err 5.9e-7 (at the f32
    noise floor of the chain) -- the runtime's ACT table loads hang, so the
    builtin Sin LUT is unusable and sin is evaluated with Square/affine ACT
    ops + DVE mul/add only.  dr chains of chunk pairs are emitted step-major
    to interleave one chunk's ACT phase with the other's DVE phase.
"""

import numpy as np

import concourse.bass as bass
import concourse.mybir as mybir
from concourse import bacc
from concourse.mybir import AluOpType as Op
from concourse.tile import TileContext

F32 = mybir.dt.float32
MAGIC = float(np.float32(12582912.0))   # 1.5 * 2^23: (x+M)-M == round(x) in f32

# sin(2*pi*u)/(2*pi) = u * P(u^2); P deg-5 minimax on [0, 0.25] (deg-11 in u,
# max err 5.9e-7 -- at the f32 noise floor of the 5-step chain)
_PC = [0.9999999403953552, -6.5797224044799805, 12.987188339233398,
       -12.195494651794434, 6.589564323425293, -2.001596689224243]

N_CORES = 8
AREAS_TOTAL = 2 * 8192
S = AREAS_TOTAL // N_CORES      # 2048 areas per core
T = S // 128                    # 16 areas per partition
CH = 2                          # areas per partition per chunk
NCHUNK = T // CH                # 8 chunks
PIX = 256                       # 16*16 pixels per area
C = 4                           # mask channels
PAD = 288                       # padded per-area mc stride (16 | 256 | 16)


def _dr_chain_multi(nc, pools, xs, width, nsteps):
    """nsteps x diff_round on a LIST of tiles, emitted step-major so the
    Tile scheduler overlaps one tile's ACT phase with another's DVE phase."""
    for _ in range(nsteps):
        xs = [_dr_step(nc, pools, x, width) for x in xs]
    return xs


def _dr_chain(nc, pools, x, width, nsteps):
    for _ in range(nsteps):
        x = _dr_step(nc, pools, x, width)
    return x


def _dr_step(nc, pools, x, width):
    """One diff_round:  x' = x - sin(2*pi*x)/(2*pi), polynomial form.

    Uses only ops verified to work on this runtime: ACT Square / Copy(scale,
    bias) and DVE tensor_scalar / tensor_tensor.
    """
    xp, ap = pools
    c = _PC
    if True:
        # All chain values live in [0,1] (up to ~1e-7 excursions), so instead
        # of u = x - round(x) use w = x - 0.5 with sin(2*pi*x) = -sin(2*pi*w):
        # dr(x) = x + w*P(w^2). One 2x tensor_scalar replaces ts+tt.
        u = ap.tile([128, width], F32, tag="sm")
        nc.vector.tensor_scalar(u[:, :], x[:, :], -0.5, None, Op.add)
        # powers on ACT
        v = ap.tile([128, width], F32, tag="sm")
        nc.scalar.activation(v[:, :], u[:, :], mybir.ActivationFunctionType.Square)
        v2 = ap.tile([128, width], F32, tag="sm")
        nc.scalar.activation(v2[:, :], v[:, :], mybir.ActivationFunctionType.Square)
        v4 = ap.tile([128, width], F32, tag="sm")
        nc.scalar.activation(v4[:, :], v2[:, :], mybir.ActivationFunctionType.Square)
        # affine groups on ACT: A=c0+c1 v, B=c2+c3 v, D=c4+c5 v
        # (GpSimd offload hangs this runtime -- POOL tensor ops appear to
        # need ucode the fake_nrt shim doesn't load -- so tensor-tensor work
        # stays on the vector engine and single-input affines go to ACT.)
        A = ap.tile([128, width], F32, tag="sm")
        nc.scalar.activation(A[:, :], v[:, :], mybir.ActivationFunctionType.Copy,
                             bias=c[0], scale=c[1])
        Bq = ap.tile([128, width], F32, tag="sm")
        nc.scalar.activation(Bq[:, :], v[:, :], mybir.ActivationFunctionType.Copy,
                             bias=c[2], scale=c[3])
        Dq = ap.tile([128, width], F32, tag="sm")
        nc.scalar.activation(Dq[:, :], v[:, :], mybir.ActivationFunctionType.Copy,
                             bias=c[4], scale=c[5])
        # P = (A + B*v2) + D*v4
        t1 = ap.tile([128, width], F32, tag="sm")
        nc.vector.tensor_tensor(t1[:, :], Bq[:, :], v2[:, :], Op.mult)
        t4 = ap.tile([128, width], F32, tag="sm")
        nc.vector.tensor_tensor(t4[:, :], Dq[:, :], v4[:, :], Op.mult)
        E = ap.tile([128, width], F32, tag="sm")
        nc.vector.tensor_tensor(E[:, :], A[:, :], t1[:, :], Op.add)
        P = ap.tile([128, width], F32, tag="sm")
        nc.vector.tensor_tensor(P[:, :], E[:, :], t4[:, :], Op.add)
        sP = ap.tile([128, width], F32, tag="sm")
        nc.vector.tensor_tensor(sP[:, :], u[:, :], P[:, :], Op.mult)
        xn = xp.tile([128, width], F32, tag=f"x{width}")
        nc.vector.tensor_tensor(xn[:, :], x[:, :], sP[:, :], Op.add)
    return xn


def build_nc():
    nc = bacc.Bacc("TRN2", target_bir_lowering=False, debug=False)

    mask_d = nc.dram_tensor("mask", [S, PIX * C], F32, kind="ExternalInput")
    img_d = nc.dram_tensor("img", [S, PIX], F32, kind="ExternalInput")
    edge_d = nc.dram_tensor("edge", [S, PIX], F32, kind="ExternalInput")
    mid_d = nc.dram_tensor("mid", [S, C], F32, kind="ExternalInput")
    me_d = nc.dram_tensor("me", [S, PIX], F32, kind="ExternalOutput")
    out2_d = nc.dram_tensor("out2", [S], F32, kind="ExternalOutput")

    # DRAM views with partition p <-> area p*T + t
    mask_v = mask_d.ap().rearrange("(p t) f -> p t f", p=128)     # [128, 16, 1024]
    img_v = img_d.ap().rearrange("(p t) f -> p t f", p=128)
    edge_v = edge_d.ap().rearrange("(p t) f -> p t f", p=128)
    mid_v = mid_d.ap().rearrange("(p t) c -> p (t c)", p=128)     # [128, 64]
    me_v = me_d.ap().rearrange("(p t) f -> p t f", p=128)
    out2_v = out2_d.ap().rearrange("(p t) -> p t", p=128)         # [128, 16]

    with TileContext(nc) as tc:
        with (
            tc.tile_pool(name="xp", bufs=5) as xp,          # dr-chain ping-pong
            tc.tile_pool(name="ap", bufs=12) as ap,         # poly scratch
            tc.tile_pool(name="zp", bufs=1) as zp,
            tc.tile_pool(name="iop", bufs=2) as iop,        # img/edge/me
            tc.tile_pool(name="smp", bufs=8) as smp,        # small work tiles
            tc.tile_pool(name="mcp", bufs=2) as mcpool,     # padded mc
            tc.tile_pool(name="stp", bufs=1) as stp,        # persistent stats
        ):
            BIG = CH * PIX * C          # 2048
            SMALL = CH * PIX            # 512

            # persistent stats tiles [128, T]
            s_mc = stp.tile([128, T], F32, tag="s_mc")
            s_mi = stp.tile([128, T], F32, tag="s_mi")
            s_me = stp.tile([128, T], F32, tag="s_me")
            s_e2 = stp.tile([128, T], F32, tag="s_e2")
            inv_t = stp.tile([128, T], F32, tag="inv")
            meann_t = stp.tile([128, T], F32, tag="meann")
            denom_t = stp.tile([128, T], F32, tag="denom")

            # mid -> m = 1 - mid, once for the whole core
            mid_t = stp.tile([128, T * C], F32, tag="mid")
            nc.sync.dma_start(mid_t[:, :], mid_v)
            m_t = stp.tile([128, T * C], F32, tag="m")
            nc.vector.tensor_scalar(m_t[:, :], mid_t[:, :], -1.0, 1.0, Op.mult, Op.add)

            for cpair in range(NCHUNK // 2):
              pair = (2 * cpair, 2 * cpair + 1)
              xs = []
              for c in pair:
                tsl = slice(c * CH, (c + 1) * CH)
                x = xp.tile([128, BIG], F32, tag=f"x{BIG}")
                nc.sync.dma_start(
                    x[:, :].rearrange("p (t f) -> p t f", t=CH), mask_v[:, tsl, :]
                )
                xs.append(x)
              # ---- 5 chained diff_round steps, pair-interleaved ----
              xs = _dr_chain_multi(nc, (xp, ap), xs, BIG, 5)
              for c, x in zip(pair, xs):
                tsl = slice(c * CH, (c + 1) * CH)
                img_t = iop.tile([128, SMALL], F32, tag="img")
                nc.sync.dma_start(
                    img_t[:, :].rearrange("p (t f) -> p t f", t=CH), img_v[:, tsl, :]
                )
                edge_t = iop.tile([128, SMALL], F32, tag="edge")
                nc.sync.dma_start(
                    edge_t[:, :].rearrange("p (t f) -> p t f", t=CH), edge_v[:, tsl, :]
                )

                # ---- flip-subtract: z = x5 - m  (m broadcast over pixels) ----
                m_b = (
                    m_t[:, c * CH * C:(c + 1) * CH * C]
                    .rearrange("p (a c) -> p a c", c=C)
                    .unsqueeze(2)
                    .to_broadcast([128, CH, PIX, C])
                )
                z = zp.tile([128, BIG], F32, tag="z")
                z4 = z[:, :].rearrange("p (a x c) -> p a x c", a=CH, c=C)
                x4 = x[:, :].rearrange("p (a x c) -> p a x c", a=CH, c=C)
                nc.vector.tensor_tensor(z4, x4, m_b, Op.subtract)

                # ---- channel-pair products, abs via sign-bit clear ----
                pa = smp.tile([128, SMALL], F32, tag="tmp")
                pb = smp.tile([128, SMALL], F32, tag="tmp")
                pa3 = pa[:, :].rearrange("p (a x) -> p a x", a=CH).unsqueeze(3)
                pb3 = pb[:, :].rearrange("p (a x) -> p a x", a=CH).unsqueeze(3)
                nc.vector.tensor_tensor(pa3, z4[:, :, :, 0:1], z4[:, :, :, 1:2], Op.mult)
                nc.vector.tensor_tensor(pb3, z4[:, :, :, 2:3], z4[:, :, :, 3:4], Op.mult)
                pa_u = pa[:, :].bitcast(mybir.dt.uint32)
                pb_u = pb[:, :].bitcast(mybir.dt.uint32)
                nc.vector.tensor_scalar(pa_u, pa_u, 0x7FFFFFFF, None, Op.bitwise_and)
                nc.vector.tensor_scalar(pb_u, pb_u, 0x7FFFFFFF, None, Op.bitwise_and)

                # ---- one more diff_round on each product ----
                wa = _dr_chain(nc, (smp, ap), pa, SMALL, 1)
                wb = _dr_chain(nc, (smp, ap), pb, SMALL, 1)

                # ---- mc = wa*wb into padded tile ----
                mcp = mcpool.tile([128, CH * PAD], F32, tag="mcp")
                mcp3 = mcp[:, :].rearrange("p (a k) -> p a k", a=CH)
                nc.vector.memset(mcp3[:, :, 0:16], 0.0)
                nc.vector.memset(mcp3[:, :, 272:288], 0.0)
                nc.vector.tensor_tensor(
                    mcp3[:, :, 16:272],
                    wa[:, :].rearrange("p (a x) -> p a x", a=CH),
                    wb[:, :].rearrange("p (a x) -> p a x", a=CH),
                    Op.mult,
                )
                nc.vector.tensor_reduce(
                    s_mc[:, tsl], mcp3[:, :, 16:272], axis=mybir.AxisListType.X,
                    op=Op.add,
                )

                mc_v = mcp3[:, :, 16:272]     # [128, CH, 256]
                up_v = mcp3[:, :, 32:288]     # mc[i+16], zero past bottom row
                dn_v = mcp3[:, :, 0:256]      # mc[i-16], zero before top row

                # ---- erosion: ev = up + dn - 2*up*dn ; r = 1 - ev*mc ----
                t1 = smp.tile([128, SMALL], F32, tag="tmp")
                p1 = smp.tile([128, SMALL], F32, tag="tmp")
                t13 = t1[:, :].rearrange("p (a x) -> p a x", a=CH)
                p13 = p1[:, :].rearrange("p (a x) -> p a x", a=CH)
                nc.vector.tensor_tensor(t13, up_v, dn_v, Op.add)
                nc.vector.tensor_tensor(p13, up_v, dn_v, Op.mult)
                p2 = smp.tile([128, SMALL], F32, tag="tmp")
                nc.vector.tensor_scalar(p2[:, :], p1[:, :], 2.0, None, Op.mult)
                ev = smp.tile([128, SMALL], F32, tag="tmp")
                nc.vector.tensor_tensor(ev[:, :], t1[:, :], p2[:, :], Op.subtract)
                q = smp.tile([128, SMALL], F32, tag="tmp")
                q3 = q[:, :].rearrange("p (a x) -> p a x", a=CH)
                nc.vector.tensor_tensor(
                    q3, ev[:, :].rearrange("p (a x) -> p a x", a=CH), mc_v, Op.mult)
                r = smp.tile([128, SMALL], F32, tag="tmp")
                nc.vector.tensor_scalar(r[:, :], q[:, :], -1.0, 1.0, Op.mult, Op.add)
                r2 = smp.tile([128, SMALL], F32, tag="tmp")
                nc.scalar.activation(r2[:, :], r[:, :],
                                     mybir.ActivationFunctionType.Square)

                # ---- masked edges me = r2 * (mc*edge) ----
                metmp = smp.tile([128, SMALL], F32, tag="tmp")
                me3 = metmp[:, :].rearrange("p (a x) -> p a x", a=CH)
                nc.vector.tensor_tensor(
                    me3, mc_v, edge_t[:, :].rearrange("p (a x) -> p a x", a=CH),
                    Op.mult)
                me_t = iop.tile([128, SMALL], F32, tag="me")
                nc.vector.tensor_tensor(me_t[:, :], r2[:, :], metmp[:, :], Op.mult)
                nc.vector.tensor_reduce(
                    s_me[:, tsl],
                    me_t[:, :].rearrange("p (a x) -> p a x", a=CH),
                    axis=mybir.AxisListType.X, op=Op.add,
                )

                # ---- masked image mi = mc * img ----
                mi = smp.tile([128, SMALL], F32, tag="tmp")
                mi3 = mi[:, :].rearrange("p (a x) -> p a x", a=CH)
                nc.vector.tensor_tensor(
                    mi3, mc_v, img_t[:, :].rearrange("p (a x) -> p a x", a=CH),
                    Op.mult)
                nc.vector.tensor_reduce(
                    s_mi[:, tsl], mi3, axis=mybir.AxisListType.X, op=Op.add,
                )

                # ---- per-chunk stats: denom, 1/denom, mean ----
                csl = tsl
                nc.vector.tensor_scalar(denom_t[:, csl], s_mc[:, csl], 1e-8, None,
                                        Op.add)
                nc.vector.reciprocal(inv_t[:, csl], denom_t[:, csl])
                nc.vector.tensor_tensor(meann_t[:, csl], s_mi[:, csl], inv_t[:, csl],
                                        Op.mult)

                # ---- variance accumulation ----
                d_t = smp.tile([128, SMALL], F32, tag="tmp")
                for j in range(CH):
                    col = c * CH + j
                    jsl = slice(j * PIX, (j + 1) * PIX)
                    nc.vector.tensor_scalar(
                        d_t[:, jsl], mi[:, jsl], meann_t[:, col:col + 1], None,
                        Op.subtract)
                e_t = smp.tile([128, SMALL], F32, tag="tmp")
                e3 = e_t[:, :].rearrange("p (a x) -> p a x", a=CH)
                nc.vector.tensor_tensor(
                    e3, d_t[:, :].rearrange("p (a x) -> p a x", a=CH), mc_v, Op.mult)
                e2_t = smp.tile([128, SMALL], F32, tag="tmp")
                nc.scalar.activation(e2_t[:, :], e_t[:, :],
                                     mybir.ActivationFunctionType.Square)
                nc.vector.tensor_reduce(
                    s_e2[:, tsl],
                    e2_t[:, :].rearrange("p (a x) -> p a x", a=CH),
                    axis=mybir.AxisListType.X, op=Op.add,
                )

                # ---- store masked edges ----
                nc.sync.dma_start(
                    me_v[:, tsl, :], me_t[:, :].rearrange("p (t f) -> p t f", t=CH)
                )

            # ---- final per-area scalar: out2 = (se2/denom) * (sme/256) * 1000
            varr = stp.tile([128, T], F32, tag="varr")
            nc.vector.tensor_tensor(varr[:, :], s_e2[:, :], inv_t[:, :], Op.mult)
            lss = stp.tile([128, T], F32, tag="lss")
            nc.vector.tensor_scalar(lss[:, :], s_me[:, :], 1000.0 / 256.0, None,
                                    Op.mult)
            o2 = stp.tile([128, T], F32, tag="o2")
            nc.vector.tensor_tensor(o2[:, :], varr[:, :], lss[:, :], Op.mult)
            nc.sync.dma_start(out2_v, o2[:, :])

    nc.compile()
    return nc


_NC_CACHE = None


def _get_nc():
    global _NC_CACHE
    if _NC_CACHE is None:
        _NC_CACHE = build_nc()
    return _NC_CACHE


def kernel(resized_image, mask_combined, initial_mask_id, edge_map,
           _results_hook=None):
    from concourse.bass_utils import run_bass_kernel_spmd

    B, A = 2, 8192
    mask_f = np.ascontiguousarray(
        np.asarray(mask_combined, dtype=np.float32).reshape(B * A, PIX * C))
    img_f = np.ascontiguousarray(
        np.asarray(resized_image, dtype=np.float32).reshape(B * A, PIX))
    edge_f = np.ascontiguousarray(
        np.asarray(edge_map, dtype=np.float32).reshape(B * A, PIX))
    mid_f = np.ascontiguousarray(
        np.asarray(initial_mask_id, dtype=np.float32).reshape(B * A, C))

    nc = _get_nc()
    in_maps = [
        {
            "mask": mask_f[i * S:(i + 1) * S],
            "img": img_f[i * S:(i + 1) * S],
            "edge": edge_f[i * S:(i + 1) * S],
            "mid": mid_f[i * S:(i + 1) * S],
        }
        for i in range(N_CORES)
    ]
    res = run_bass_kernel_spmd(nc, in_maps, core_ids=list(range(N_CORES)))
    if _results_hook is not None:
        _results_hook(res)

    me = np.concatenate([r["me"] for r in res.results], axis=0)
    out2 = np.concatenate([r["out2"] for r in res.results], axis=0)
    return (
        me.reshape(B, A, 16, 16, 1).astype(np.float32),
        out2.reshape(B, A).astype(np.float32),
    )


# revision 24
# speedup vs baseline: 1.6304x; 1.1897x over previous
"""Trainium2 Bass kernel for the batched multi-mask de-conv loss problem.

Computes, per (batch, area) over [B=2, A=8192] independent 16x16 areas:
  mc     = differentiable mask-of-interest from mask_combined vs initial_mask_id
  eroded = soft erosion of mc (vertical neighbours only -- or_simple(a,b)=a(2-a)
           makes the horizontal branch algebraically dead)
  me     = eroded * edge_map                      -> output[..., None]
  out2   = var(masked image) * mean(me) * 1000    -> per-area scalar

Sharding: fully data-parallel over B*A = 16384 areas; 2048 areas per core on
8 NeuronCores, SPMD (identical program, different data), no collectives.

Key math identities used (exact up to f32 rounding noise ~1e-6):
  - b = harder_diff_round(mid) == mid exactly for mid in {0,1}
  - eq-select: agree = hdr(a) if b==1 else 1-hdr(a) = |hdr(a) - (1-b)|
  - dr(x - m) = dr(x) - m for integer m; dr(|x|) = |dr(x)| (dr is odd around
    integers) -> the whole per-channel pipeline collapses to 5 chained
    diff_round steps on the raw mask, followed by one flip-subtract.
  - diff_round via a degree-11 odd minimax polynomial of the wrapped residue
    u = x - round(x):  sin(2*pi*x) = u * P(u^2), max err 5.9e-7 (at the f32
    noise floor of the chain) -- the runtime's ACT table loads hang, so the
    builtin Sin LUT is unusable and sin is evaluated with Square/affine ACT
    ops + DVE mul/add only.  dr chains of chunk pairs are emitted step-major
    to interleave one chunk's ACT phase with the other's DVE phase.
"""

import numpy as np

import concourse.bass as bass
import concourse.mybir as mybir
from concourse import bacc
from concourse.mybir import AluOpType as Op
from concourse.tile import TileContext

F32 = mybir.dt.float32
MAGIC = float(np.float32(12582912.0))   # 1.5 * 2^23: (x+M)-M == round(x) in f32

# sin(2*pi*u)/(2*pi) = u * P(u^2); P deg-5 minimax on [0, 0.25] (deg-11 in u,
# max err 5.9e-7 -- at the f32 noise floor of the 5-step chain)
_PC = [0.9999999403953552, -6.5797224044799805, 12.987188339233398,
       -12.195494651794434, 6.589564323425293, -2.001596689224243]

N_CORES = 8
AREAS_TOTAL = 2 * 8192
S = AREAS_TOTAL // N_CORES      # 2048 areas per core
T = S // 128                    # 16 areas per partition
CH = 2                          # areas per partition per chunk
NCHUNK = T // CH                # 8 chunks
PIX = 256                       # 16*16 pixels per area
C = 4                           # mask channels
PAD = 288                       # padded per-area mc stride (16 | 256 | 16)


def _dr_chain_multi(nc, pools, xs, width, nsteps):
    """nsteps x diff_round on a LIST of tiles, emitted step-major so the
    Tile scheduler overlaps one tile's ACT phase with another's DVE phase."""
    for _ in range(nsteps):
        xs = [_dr_step(nc, pools, x, width) for x in xs]
    return xs


def _dr_chain(nc, pools, x, width, nsteps):
    for _ in range(nsteps):
        x = _dr_step(nc, pools, x, width)
    return x


def _dr_step(nc, pools, x, width):
    """One diff_round:  x' = x - sin(2*pi*x)/(2*pi), polynomial form.

    Uses only ops verified to work on this runtime: ACT Square / Copy(scale,
    bias) and DVE tensor_scalar / tensor_tensor.
    """
    xp, ap = pools
    c = _PC
    if True:
        # All chain values live in [0,1] (up to ~1e-7 excursions), so instead
        # of u = x - round(x) use w = x - 0.5 with sin(2*pi*x) = -sin(2*pi*w):
        # dr(x) = x + w*P(w^2). One 2x tensor_scalar replaces ts+tt.
        u = ap.tile([128, width], F32, tag="sm")
        nc.vector.tensor_scalar(u[:, :], x[:, :], -0.5, None, Op.add)
        # powers on ACT
        v = ap.tile([128, width], F32, tag="sm")
        nc.scalar.activation(v[:, :], u[:, :], mybir.ActivationFunctionType.Square)
        v2 = ap.tile([128, width], F32, tag="sm")
        nc.scalar.activation(v2[:, :], v[:, :], mybir.ActivationFunctionType.Square)
        v4 = ap.tile([128, width], F32, tag="sm")
        nc.scalar.activation(v4[:, :], v2[:, :], mybir.ActivationFunctionType.Square)
        # affine groups on ACT: A=c0+c1 v, B=c2+c3 v, D=c4+c5 v
        # (GpSimd offload hangs this runtime -- POOL tensor ops appear to
        # need ucode the fake_nrt shim doesn't load -- so tensor-tensor work
        # stays on the vector engine and single-input affines go to ACT.)
        A = ap.tile([128, width], F32, tag="sm")
        nc.scalar.activation(A[:, :], v[:, :], mybir.ActivationFunctionType.Copy,
                             bias=c[0], scale=c[1])
        Bq = ap.tile([128, width], F32, tag="sm")
        nc.scalar.activation(Bq[:, :], v[:, :], mybir.ActivationFunctionType.Copy,
                             bias=c[2], scale=c[3])
        Dq = ap.tile([128, width], F32, tag="sm")
        nc.scalar.activation(Dq[:, :], v[:, :], mybir.ActivationFunctionType.Copy,
                             bias=c[4], scale=c[5])
        # P = (A + B*v2) + D*v4
        t1 = ap.tile([128, width], F32, tag="sm")
        nc.vector.tensor_tensor(t1[:, :], Bq[:, :], v2[:, :], Op.mult)
        nc.vector.tensor_tensor(Dq[:, :], Dq[:, :], v4[:, :], Op.mult)   # t4 in-place
        nc.vector.tensor_tensor(A[:, :], A[:, :], t1[:, :], Op.add)      # E in-place
        nc.vector.tensor_tensor(A[:, :], A[:, :], Dq[:, :], Op.add)      # P in-place
        nc.vector.tensor_tensor(u[:, :], u[:, :], A[:, :], Op.mult)      # sP in-place
        xn = xp.tile([128, width], F32, tag=f"x{width}")
        nc.vector.tensor_tensor(xn[:, :], x[:, :], u[:, :], Op.add)
    return xn


def build_nc():
    nc = bacc.Bacc("TRN2", target_bir_lowering=False, debug=False)

    mask_d = nc.dram_tensor("mask", [S, PIX * C], F32, kind="ExternalInput")
    img_d = nc.dram_tensor("img", [S, PIX], F32, kind="ExternalInput")
    edge_d = nc.dram_tensor("edge", [S, PIX], F32, kind="ExternalInput")
    mid_d = nc.dram_tensor("mid", [S, C], F32, kind="ExternalInput")
    me_d = nc.dram_tensor("me", [S, PIX], F32, kind="ExternalOutput")
    out2_d = nc.dram_tensor("out2", [S], F32, kind="ExternalOutput")

    # DRAM views with partition p <-> area p*T + t
    mask_v = mask_d.ap().rearrange("(p t) f -> p t f", p=128)     # [128, 16, 1024]
    img_v = img_d.ap().rearrange("(p t) f -> p t f", p=128)
    edge_v = edge_d.ap().rearrange("(p t) f -> p t f", p=128)
    mid_v = mid_d.ap().rearrange("(p t) c -> p (t c)", p=128)     # [128, 64]
    me_v = me_d.ap().rearrange("(p t) f -> p t f", p=128)
    out2_v = out2_d.ap().rearrange("(p t) -> p t", p=128)         # [128, 16]

    with TileContext(nc) as tc:
        with (
            tc.tile_pool(name="xp", bufs=5) as xp,          # dr-chain ping-pong
            tc.tile_pool(name="ap", bufs=12) as ap,         # poly scratch
            tc.tile_pool(name="zp", bufs=1) as zp,
            tc.tile_pool(name="iop", bufs=2) as iop,        # img/edge/me
            tc.tile_pool(name="smp", bufs=8) as smp,        # small work tiles
            tc.tile_pool(name="mcp", bufs=2) as mcpool,     # padded mc
            tc.tile_pool(name="stp", bufs=1) as stp,        # persistent stats
        ):
            BIG = CH * PIX * C          # 2048
            SMALL = CH * PIX            # 512

            # persistent stats tiles [128, T]
            s_mc = stp.tile([128, T], F32, tag="s_mc")
            s_mi = stp.tile([128, T], F32, tag="s_mi")
            s_me = stp.tile([128, T], F32, tag="s_me")
            s_e2 = stp.tile([128, T], F32, tag="s_e2")
            inv_t = stp.tile([128, T], F32, tag="inv")
            meann_t = stp.tile([128, T], F32, tag="meann")
            denom_t = stp.tile([128, T], F32, tag="denom")

            # mid -> m = 1 - mid, once for the whole core
            mid_t = stp.tile([128, T * C], F32, tag="mid")
            nc.sync.dma_start(mid_t[:, :], mid_v)
            m_t = stp.tile([128, T * C], F32, tag="m")
            nc.vector.tensor_scalar(m_t[:, :], mid_t[:, :], -1.0, 1.0, Op.mult, Op.add)

            for cpair in range(NCHUNK // 2):
              pair = (2 * cpair, 2 * cpair + 1)
              xs = []
              for c in pair:
                tsl = slice(c * CH, (c + 1) * CH)
                x = xp.tile([128, BIG], F32, tag=f"x{BIG}")
                nc.sync.dma_start(
                    x[:, :].rearrange("p (t f) -> p t f", t=CH), mask_v[:, tsl, :]
                )
                xs.append(x)
              # ---- 5 chained diff_round steps, pair-interleaved ----
              xs = _dr_chain_multi(nc, (xp, ap), xs, BIG, 5)
              for c, x in zip(pair, xs):
                tsl = slice(c * CH, (c + 1) * CH)
                img_t = iop.tile([128, SMALL], F32, tag="img")
                nc.sync.dma_start(
                    img_t[:, :].rearrange("p (t f) -> p t f", t=CH), img_v[:, tsl, :]
                )
                edge_t = iop.tile([128, SMALL], F32, tag="edge")
                nc.sync.dma_start(
                    edge_t[:, :].rearrange("p (t f) -> p t f", t=CH), edge_v[:, tsl, :]
                )

                # ---- flip-subtract: z = x5 - m  (m broadcast over pixels) ----
                m_b = (
                    m_t[:, c * CH * C:(c + 1) * CH * C]
                    .rearrange("p (a c) -> p a c", c=C)
                    .unsqueeze(2)
                    .to_broadcast([128, CH, PIX, C])
                )
                z = zp.tile([128, BIG], F32, tag="z")
                z4 = z[:, :].rearrange("p (a x c) -> p a x c", a=CH, c=C)
                x4 = x[:, :].rearrange("p (a x c) -> p a x c", a=CH, c=C)
                nc.vector.tensor_tensor(z4, x4, m_b, Op.subtract)

                # ---- channel-pair products, abs via sign-bit clear ----
                pa = smp.tile([128, SMALL], F32, tag="tmp")
                pb = smp.tile([128, SMALL], F32, tag="tmp")
                pa3 = pa[:, :].rearrange("p (a x) -> p a x", a=CH).unsqueeze(3)
                pb3 = pb[:, :].rearrange("p (a x) -> p a x", a=CH).unsqueeze(3)
                nc.vector.tensor_tensor(pa3, z4[:, :, :, 0:1], z4[:, :, :, 1:2], Op.mult)
                nc.vector.tensor_tensor(pb3, z4[:, :, :, 2:3], z4[:, :, :, 3:4], Op.mult)
                pa_u = pa[:, :].bitcast(mybir.dt.uint32)
                pb_u = pb[:, :].bitcast(mybir.dt.uint32)
                nc.vector.tensor_scalar(pa_u, pa_u, 0x7FFFFFFF, None, Op.bitwise_and)
                nc.vector.tensor_scalar(pb_u, pb_u, 0x7FFFFFFF, None, Op.bitwise_and)

                # ---- one more diff_round on each product ----
                wa = _dr_chain(nc, (smp, ap), pa, SMALL, 1)
                wb = _dr_chain(nc, (smp, ap), pb, SMALL, 1)

                # ---- mc = wa*wb into padded tile ----
                mcp = mcpool.tile([128, CH * PAD], F32, tag="mcp")
                mcp3 = mcp[:, :].rearrange("p (a k) -> p a k", a=CH)
                nc.vector.memset(mcp3[:, :, 0:16], 0.0)
                nc.vector.memset(mcp3[:, :, 272:288], 0.0)
                nc.vector.tensor_tensor(
                    mcp3[:, :, 16:272],
                    wa[:, :].rearrange("p (a x) -> p a x", a=CH),
                    wb[:, :].rearrange("p (a x) -> p a x", a=CH),
                    Op.mult,
                )
                nc.vector.tensor_reduce(
                    s_mc[:, tsl], mcp3[:, :, 16:272], axis=mybir.AxisListType.X,
                    op=Op.add,
                )

                mc_v = mcp3[:, :, 16:272]     # [128, CH, 256]
                up_v = mcp3[:, :, 32:288]     # mc[i+16], zero past bottom row
                dn_v = mcp3[:, :, 0:256]      # mc[i-16], zero before top row

                # ---- erosion: ev = up + dn - 2*up*dn ; r = 1 - ev*mc ----
                t1 = smp.tile([128, SMALL], F32, tag="tmp")
                p1 = smp.tile([128, SMALL], F32, tag="tmp")
                t13 = t1[:, :].rearrange("p (a x) -> p a x", a=CH)
                p13 = p1[:, :].rearrange("p (a x) -> p a x", a=CH)
                nc.vector.tensor_tensor(t13, up_v, dn_v, Op.add)
                nc.vector.tensor_tensor(p13, up_v, dn_v, Op.mult)
                p2 = smp.tile([128, SMALL], F32, tag="tmp")
                nc.vector.tensor_scalar(p2[:, :], p1[:, :], 2.0, None, Op.mult)
                ev = smp.tile([128, SMALL], F32, tag="tmp")
                nc.vector.tensor_tensor(ev[:, :], t1[:, :], p2[:, :], Op.subtract)
                q = smp.tile([128, SMALL], F32, tag="tmp")
                q3 = q[:, :].rearrange("p (a x) -> p a x", a=CH)
                nc.vector.tensor_tensor(
                    q3, ev[:, :].rearrange("p (a x) -> p a x", a=CH), mc_v, Op.mult)
                r = smp.tile([128, SMALL], F32, tag="tmp")
                nc.vector.tensor_scalar(r[:, :], q[:, :], -1.0, 1.0, Op.mult, Op.add)
                r2 = smp.tile([128, SMALL], F32, tag="tmp")
                nc.scalar.activation(r2[:, :], r[:, :],
                                     mybir.ActivationFunctionType.Square)

                # ---- masked edges me = r2 * (mc*edge) ----
                metmp = smp.tile([128, SMALL], F32, tag="tmp")
                me3 = metmp[:, :].rearrange("p (a x) -> p a x", a=CH)
                nc.vector.tensor_tensor(
                    me3, mc_v, edge_t[:, :].rearrange("p (a x) -> p a x", a=CH),
                    Op.mult)
                me_t = iop.tile([128, SMALL], F32, tag="me")
                nc.vector.tensor_tensor(me_t[:, :], r2[:, :], metmp[:, :], Op.mult)
                nc.vector.tensor_reduce(
                    s_me[:, tsl],
                    me_t[:, :].rearrange("p (a x) -> p a x", a=CH),
                    axis=mybir.AxisListType.X, op=Op.add,
                )

                # ---- masked image mi = mc * img ----
                mi = smp.tile([128, SMALL], F32, tag="tmp")
                mi3 = mi[:, :].rearrange("p (a x) -> p a x", a=CH)
                nc.vector.tensor_tensor(
                    mi3, mc_v, img_t[:, :].rearrange("p (a x) -> p a x", a=CH),
                    Op.mult)
                nc.vector.tensor_reduce(
                    s_mi[:, tsl], mi3, axis=mybir.AxisListType.X, op=Op.add,
                )

                # ---- per-chunk stats: denom, 1/denom, mean ----
                csl = tsl
                nc.vector.tensor_scalar(denom_t[:, csl], s_mc[:, csl], 1e-8, None,
                                        Op.add)
                nc.vector.reciprocal(inv_t[:, csl], denom_t[:, csl])
                nc.vector.tensor_tensor(meann_t[:, csl], s_mi[:, csl], inv_t[:, csl],
                                        Op.mult)

                # ---- variance accumulation ----
                d_t = smp.tile([128, SMALL], F32, tag="tmp")
                for j in range(CH):
                    col = c * CH + j
                    jsl = slice(j * PIX, (j + 1) * PIX)
                    nc.vector.tensor_scalar(
                        d_t[:, jsl], mi[:, jsl], meann_t[:, col:col + 1], None,
                        Op.subtract)
                e_t = smp.tile([128, SMALL], F32, tag="tmp")
                e3 = e_t[:, :].rearrange("p (a x) -> p a x", a=CH)
                nc.vector.tensor_tensor(
                    e3, d_t[:, :].rearrange("p (a x) -> p a x", a=CH), mc_v, Op.mult)
                e2_t = smp.tile([128, SMALL], F32, tag="tmp")
                nc.scalar.activation(e2_t[:, :], e_t[:, :],
                                     mybir.ActivationFunctionType.Square)
                nc.vector.tensor_reduce(
                    s_e2[:, tsl],
                    e2_t[:, :].rearrange("p (a x) -> p a x", a=CH),
                    axis=mybir.AxisListType.X, op=Op.add,
                )

                # ---- store masked edges ----
                nc.sync.dma_start(
                    me_v[:, tsl, :], me_t[:, :].rearrange("p (t f) -> p t f", t=CH)
                )

            # ---- final per-area scalar: out2 = (se2/denom) * (sme/256) * 1000
            varr = stp.tile([128, T], F32, tag="varr")
            nc.vector.tensor_tensor(varr[:, :], s_e2[:, :], inv_t[:, :], Op.mult)
            lss = stp.tile([128, T], F32, tag="lss")
            nc.vector.tensor_scalar(lss[:, :], s_me[:, :], 1000.0 / 256.0, None,
                                    Op.mult)
            o2 = stp.tile([128, T], F32, tag="o2")
            nc.vector.tensor_tensor(o2[:, :], varr[:, :], lss[:, :], Op.mult)
            nc.sync.dma_start(out2_v, o2[:, :])

    nc.compile()
    return nc


_NC_CACHE = None


def _get_nc():
    global _NC_CACHE
    if _NC_CACHE is None:
        _NC_CACHE = build_nc()
    return _NC_CACHE


def kernel(resized_image, mask_combined, initial_mask_id, edge_map,
           _results_hook=None):
    from concourse.bass_utils import run_bass_kernel_spmd

    B, A = 2, 8192
    mask_f = np.ascontiguousarray(
        np.asarray(mask_combined, dtype=np.float32).reshape(B * A, PIX * C))
    img_f = np.ascontiguousarray(
        np.asarray(resized_image, dtype=np.float32).reshape(B * A, PIX))
    edge_f = np.ascontiguousarray(
        np.asarray(edge_map, dtype=np.float32).reshape(B * A, PIX))
    mid_f = np.ascontiguousarray(
        np.asarray(initial_mask_id, dtype=np.float32).reshape(B * A, C))

    nc = _get_nc()
    in_maps = [
        {
            "mask": mask_f[i * S:(i + 1) * S],
            "img": img_f[i * S:(i + 1) * S],
            "edge": edge_f[i * S:(i + 1) * S],
            "mid": mid_f[i * S:(i + 1) * S],
        }
        for i in range(N_CORES)
    ]
    res = run_bass_kernel_spmd(nc, in_maps, core_ids=list(range(N_CORES)))
    if _results_hook is not None:
        _results_hook(res)

    me = np.concatenate([r["me"] for r in res.results], axis=0)
    out2 = np.concatenate([r["out2"] for r in res.results], axis=0)
    return (
        me.reshape(B, A, 16, 16, 1).astype(np.float32),
        out2.reshape(B, A).astype(np.float32),
    )
